# revision 38
# baseline (speedup 1.0000x reference)
"""Bahdanau attention kernel for 8 Trainium2 NeuronCores.

Full shapes: encoder_outputs [64, 2048, 512] f32, decoder_hidden [64, 512] f32,
encoder_mask [64, 2048] i32, W_enc [256, 512], W_dec [256, 512], v [256].
Outputs: context [64, 512] f32, weights [64, 2048] f32.

Sharding: data-parallel over batch, 8 batches per core, SPMD (same program,
different data slices), no collectives.

Math trick: dec_feat = dec @ W_dec^T is folded into the encoder features on
the host.  x_b = W_enc^+ dec_feat_b solves W_enc @ x_b = dec_feat_b exactly
(W_enc has full row rank 256 < 512), so
    tanh(W_enc @ (enc^T + x_b 1^T)) = tanh(enc_feat + dec_feat)
matches the reference up to float rounding.  The x-add rides the PSUM->SBUF
eviction of the PE transposes for free.
"""

import os

import numpy as np

B, S, E, D, A = 64, 2048, 512, 512, 256
NCORES = 8
BC = B // NCORES  # 8 batches per core
P = 128
SJ = S // P       # 16 s-blocks of 128 per batch
ET = E // P       # 4 e-chunks of 128
JG = 4            # s-blocks per transpose group

_cached = {}


def _build_program(stage=99):
    from contextlib import ExitStack

    import concourse.bacc as bacc
    import concourse.bass as bass
    import concourse.mybir as mybir
    import concourse.tile as tile
    from concourse.masks import make_identity

    f32 = mybir.dt.float32
    f32r = mybir.dt.float32r
    f16 = mybir.dt.float16
    i32 = mybir.dt.int32
    Tanh = mybir.ActivationFunctionType.Tanh
    Exp = mybir.ActivationFunctionType.Exp
    PSUM = bass.MemorySpace.PSUM

    nc = bacc.Bacc(
        "TRN2",
        target_bir_lowering=False,
        debug=False,
        enable_asserts=False,
        num_devices=NCORES,
    )

    enc = nc.dram_tensor("enc", [BC, S, E], f32r, kind="ExternalInput").ap()
    wenc = nc.dram_tensor("wenc", [A, E], f32, kind="ExternalInput").ap()
    xcor = nc.dram_tensor("xcor", [BC, E], f32, kind="ExternalInput").ap()
    vin = nc.dram_tensor("vin", [A], f32, kind="ExternalInput").ap()
    maskin = nc.dram_tensor("maskin", [BC, S], i32, kind="ExternalInput").ap()
    ctx_out = nc.dram_tensor("ctx", [BC, E], f32, kind="ExternalOutput").ap()
    w_out = nc.dram_tensor("wout", [BC, S], f32, kind="ExternalOutput").ap()

    with tile.TileContext(nc) as tc:
        with ExitStack() as ctx:
            consts = ctx.enter_context(tc.tile_pool(name="consts", bufs=1))
            enc_pool = ctx.enter_context(tc.tile_pool(name="enc", bufs=3))
            encT_pool = ctx.enter_context(tc.tile_pool(name="encT", bufs=2))
            tanh_pool = ctx.enter_context(tc.tile_pool(name="tanh", bufs=4))
            scratch_pool = ctx.enter_context(tc.tile_pool(name="scr", bufs=3))
            small_pool = ctx.enter_context(tc.tile_pool(name="small", bufs=4))
            psumT_pool = ctx.enter_context(
                tc.tile_pool(name="psumT", bufs=3, space=PSUM)
            )
            psumS_pool = ctx.enter_context(
                tc.tile_pool(name="psumS", bufs=2, space=PSUM)
            )
            psumM_pool = ctx.enter_context(
                tc.tile_pool(name="psumM", bufs=1, space=PSUM)
            )

            # ---------------- setup (once) ----------------
            ident = consts.tile([P, P], f32)
            make_identity(nc, ident)
            ident_r = consts.tile([P, P], f32r)
            nc.vector.tensor_copy(ident_r, ident)

            ones_mat = consts.tile([P, P], f32)
            nc.vector.memset(ones_mat, 1.0)

            if stage >= 1:
                # v replicated across partitions (and twice along free): [128, 2, 256]
                v_rep_f = consts.tile([P, 4, A], f32)
                nc.gpsimd.dma_start(
                    out=v_rep_f,
                    in_=bass.AP(
                        tensor=vin.tensor, offset=0, ap=[[0, P], [0, 4], [1, A]]
                    ),
                )
                v_rep = consts.tile([P, 4, A], f16)
                nc.vector.tensor_copy(v_rep, v_rep_f)

                # W_enc^T in SBUF: [128(e_lo), 4(e_chunk), 256(a)]
                wenc_nat = consts.tile([P, 2, E], f32)
                nc.sync.dma_start(
                    out=wenc_nat, in_=wenc.rearrange("(ah p) e -> p ah e", p=P)
                )
                wencT = consts.tile([P, ET, A], f16)
                for ah in range(2):
                    for t in range(ET):
                        ps = psumM_pool.tile([P, P], f32, tag="psmisc")
                        nc.tensor.transpose(
                            ps, wenc_nat[:, ah, t * P : (t + 1) * P], ident
                        )
                        nc.vector.tensor_copy(
                            wencT[:, t, ah * P : (ah + 1) * P], ps
                        )

                # x correction, transposed to [128(e_lo), 4(e_chunk), 8(b)]
                x_nat = consts.tile([BC, E], f32)
                nc.sync.dma_start(out=x_nat, in_=xcor)
                xT = consts.tile([P, ET, BC], f32)
                for t in range(ET):
                    ps = psumM_pool.tile([P, BC], f32, tag="psmisc")
                    nc.tensor.transpose(
                        ps, x_nat[:, t * P : (t + 1) * P], ident[:BC, :BC]
                    )
                    nc.vector.tensor_copy(xT[:, t, :], ps)

                # mask -> additive -BIG mask, transposed to [128(s_lo), 16(j), 8(b)]
                mask_nat = consts.tile([BC, S], i32)
                nc.sync.dma_start(out=mask_nat, in_=maskin)
                mask_f = consts.tile([BC, S], f32)
                nc.vector.tensor_copy(mask_f, mask_nat)
                mask_m1 = consts.tile([BC, S], f32)
                nc.vector.tensor_scalar(
                    out=mask_m1,
                    in0=mask_f,
                    scalar1=1.0,
                    scalar2=1.0e30,
                    op0=mybir.AluOpType.subtract,
                    op1=mybir.AluOpType.mult,
                )
                maskT = consts.tile([P, SJ, BC], f32)
                for j in range(SJ):
                    ps = psumM_pool.tile([P, BC], f32, tag="psmisc")
                    nc.tensor.transpose(
                        ps, mask_m1[:, j * P : (j + 1) * P], ident[:BC, :BC]
                    )
                    nc.vector.tensor_copy(maskT[:, j, :], ps)

            # ---------------- per-batch pipeline ----------------
            # Front half (loads, transposes, scores matmuls, tanh, v-dot) of
            # batch b is emitted before the back half (softmax, context,
            # outputs) of batch b-1, so the PE's in-order stream never stalls
            # waiting on the exp->Z->context chain: the back-half matmuls of
            # b-1 sit behind the dense transpose/scores work of b.
            JQ = 4  # j-quad granularity for tanh / v-dot

            def emit_front(b):
                enc_sb = enc_pool.tile([P, SJ, E], f32r, tag="enc")
                csz = 2 if b == 0 else JG  # finer first-batch chunks: earlier start
                for c0 in range(0, SJ, csz):
                    nc.sync.dma_start(
                        out=enc_sb[:, c0 : c0 + csz, :],
                        in_=enc[b].rearrange("(j p) e -> p j e", p=P)[
                            :, c0 : c0 + csz, :
                        ],
                    )
                scores_sb = small_pool.tile([P, SJ], f32, tag="scores")

                # Phase 1: all 64 transposes of the batch, then Phase 2: all
                # 64 scores matmuls (this ordering benches faster than
                # interleaving per j-group)
                encT_b = encT_pool.tile([P, ET, SJ, P], f16, tag="encT")
                for jg in range(SJ // JG):
                    for t in range(ET):
                        psT = psumT_pool.tile([P, JG, P], f32r, tag="psT")
                        for jj in range(JG):
                            nc.tensor.transpose(
                                psT[:, jj, :],
                                enc_sb[:, jg * JG + jj, t * P : (t + 1) * P],
                                ident_r,
                            )
                        # evict + x-correction, rounding to fp16 so the scores
                        # matmuls get a 2-byte stationary (fast weight load)
                        if t >= 2:
                            nc.vector.tensor_scalar_add(
                                encT_b[:, t, jg * JG : (jg + 1) * JG, :],
                                psT,
                                xT[:, t, b : b + 1],
                            )
                        else:
                            nc.scalar.add(
                                encT_b[:, t, jg * JG : (jg + 1) * JG, :],
                                psT,
                                xT[:, t, b : b + 1],
                            )

                for jg in range(SJ // JG):
                    psS = psumS_pool.tile([P, JQ, A], f32, tag="psS")
                    for jl in range(JQ):
                        jglob = jg * JG + jl
                        for t in range(ET):
                            nc.tensor.matmul(
                                psS[:, jl, :],
                                lhsT=encT_b[:, t, jglob, :],
                                rhs=wencT[:, t, :],
                                start=(jl % 2 == 0 and t == 0),
                                stop=(jl % 2 == 1 and t == ET - 1),
                            )
                    tanh_sb = tanh_pool.tile([P, JQ, A], f16, tag="tanh")
                    nc.scalar.activation(tanh_sb, psS, Tanh)
                    jglob = jg * JG
                    scr = scratch_pool.tile([P, JQ, A], f16, tag="scr")
                    nc.vector.tensor_mul(scr, tanh_sb, v_rep)
                    nc.vector.tensor_reduce(
                        out=scores_sb[:, jglob : jglob + JQ],
                        in_=scr,
                        axis=mybir.AxisListType.X,
                        op=mybir.AluOpType.add,
                    )
                return enc_sb, scores_sb

            def emit_back(b, enc_sb, scores_sb):
                # mask + exp (+ per-partition partial sums)
                scores_m = small_pool.tile([P, SJ], f32, tag="scoresm")
                nc.vector.tensor_add(scores_m, scores_sb, maskT[:, :, b])
                exp_sb = small_pool.tile([P, SJ], f32r, tag="exp")
                zpart = small_pool.tile([P, 1], f32, tag="zpart")
                nc.scalar.activation(exp_sb, scores_m, Exp, accum_out=zpart)

                # Z = sum over partitions via all-ones matmul -> [128, 1]
                psZ = psumM_pool.tile([P, 1], f32, tag="psmisc")
                nc.tensor.matmul(psZ, lhsT=ones_mat, rhs=zpart, start=True, stop=True)
                recip_rep = small_pool.tile([P, 1], f32, tag="reciprep")
                nc.vector.reciprocal(recip_rep, psZ)

                # context = (sum_s w_un[s] * enc[s, :]) * recip
                psC = psumM_pool.tile([1, E], f32, tag="psmisc")
                for j in range(SJ):
                    nc.tensor.matmul(
                        psC,
                        lhsT=exp_sb[:, j : j + 1],
                        rhs=enc_sb[:, j, :],
                        start=(j == 0),
                        stop=(j == SJ - 1),
                    )
                ctx_sb = small_pool.tile([1, E], f32, tag="ctxsb")
                nc.vector.tensor_scalar_mul(ctx_sb, psC, recip_rep[0:1, :])
                nc.sync.dma_start(out=ctx_out[b : b + 1, :], in_=ctx_sb)

                # normalized weights, transposed back to s-major for output
                wn = small_pool.tile([P, SJ], f32, tag="wn")
                nc.vector.tensor_scalar_mul(wn, exp_sb, recip_rep)
                psW = psumM_pool.tile([SJ, P], f32, tag="psmisc")
                nc.tensor.transpose(psW, wn, ident)
                wt_sb = small_pool.tile([SJ, P], f32, tag="wt")
                nc.vector.tensor_copy(wt_sb, psW)
                nc.sync.dma_start(
                    out=w_out[b].rearrange("(j x) -> j x", x=P), in_=wt_sb
                )

            pend = None
            for b in range(BC):
                front = emit_front(b)
                if pend is not None:
                    emit_back(pend[0], *pend[1])
                pend = (b, front)
            emit_back(pend[0], *pend[1])

    nc.compile()
    return nc


def _get_program():
    stage = int(os.environ.get("KERNEL_STAGE", "99"))
    key = ("nc", stage)
    if key not in _cached:
        _cached[key] = _build_program(stage)
    return _cached[key]


def kernel(encoder_outputs, decoder_hidden, encoder_mask, W_enc, W_dec, v):
    # Recover wedged NeuronCores from any earlier crashed run.
    os.environ.setdefault("NEURON_RT_RESET_CORES", "1")
    from concourse.bass_utils import run_bass_kernel_spmd

    enc = np.ascontiguousarray(encoder_outputs, dtype=np.float32)
    dec = np.asarray(decoder_hidden, dtype=np.float32)
    mask = np.ascontiguousarray(encoder_mask, dtype=np.int32)
    we = np.ascontiguousarray(W_enc, dtype=np.float32)
    wd = np.asarray(W_dec, dtype=np.float32)
    vv = np.ascontiguousarray(v, dtype=np.float32)

    # host-side: dec_feat, then x = W_enc^+ dec_feat (exact: full row rank)
    dec_feat = dec.astype(np.float64) @ wd.astype(np.float64).T  # [B, A]
    G = we.astype(np.float64)
    try:
        x_all = (G.T @ np.linalg.solve(G @ G.T, dec_feat.T)).T  # [B, E]
    except np.linalg.LinAlgError:
        x_all = np.linalg.lstsq(G, dec_feat.T, rcond=None)[0].T
    x_all = np.ascontiguousarray(x_all, dtype=np.float32)

    nc = _get_program()
    in_maps = []
    for c in range(NCORES):
        sl = slice(c * BC, (c + 1) * BC)
        in_maps.append(
            {
                "enc": enc[sl],
                "wenc": we,
                "xcor": x_all[sl],
                "vin": vv,
                "maskin": mask[sl],
            }
        )

    trace = bool(int(os.environ.get("KERNEL_TRACE", "0")))
    res = run_bass_kernel_spmd(
        nc, in_maps, core_ids=list(range(NCORES)), trace=trace
    )
    _cached["last_results"] = res

    context = np.concatenate([r["ctx"] for r in res.results], axis=0)
    weights = np.concatenate([r["wout"] for r in res.results], axis=0)
    return context, weights


# revision 39
# speedup vs baseline: 1.0260x; 1.0260x over previous
"""Bahdanau attention kernel for 8 Trainium2 NeuronCores.

Full shapes: encoder_outputs [64, 2048, 512] f32, decoder_hidden [64, 512] f32,
encoder_mask [64, 2048] i32, W_enc [256, 512], W_dec [256, 512], v [256].
Outputs: context [64, 512] f32, weights [64, 2048] f32.

Sharding: data-parallel over batch, 8 batches per core, SPMD (same program,
different data slices), no collectives.

Math trick: dec_feat = dec @ W_dec^T is folded into the encoder features on
the host.  x_b = W_enc^+ dec_feat_b solves W_enc @ x_b = dec_feat_b exactly
(W_enc has full row rank 256 < 512), so
    tanh(W_enc @ (enc^T + x_b 1^T)) = tanh(enc_feat + dec_feat)
matches the reference up to float rounding.  The x-add rides the PSUM->SBUF
eviction of the PE transposes for free.
"""

import os

import numpy as np

B, S, E, D, A = 64, 2048, 512, 512, 256
NCORES = 8
BC = B // NCORES  # 8 batches per core
P = 128
SJ = S // P       # 16 s-blocks of 128 per batch
ET = E // P       # 4 e-chunks of 128
JG = 4            # s-blocks per transpose group

_cached = {}


def _build_program(stage=99):
    from contextlib import ExitStack

    import concourse.bacc as bacc
    import concourse.bass as bass
    import concourse.mybir as mybir
    import concourse.tile as tile
    from concourse.masks import make_identity

    f32 = mybir.dt.float32
    f32r = mybir.dt.float32r
    f16 = mybir.dt.float16
    i32 = mybir.dt.int32
    Tanh = mybir.ActivationFunctionType.Tanh
    Exp = mybir.ActivationFunctionType.Exp
    PSUM = bass.MemorySpace.PSUM

    nc = bacc.Bacc(
        "TRN2",
        target_bir_lowering=False,
        debug=False,
        enable_asserts=False,
        num_devices=NCORES,
    )

    enc = nc.dram_tensor("enc", [BC, S, E], f32r, kind="ExternalInput").ap()
    wenc = nc.dram_tensor("wenc", [A, E], f32, kind="ExternalInput").ap()
    xcor = nc.dram_tensor("xcor", [BC, E], f32, kind="ExternalInput").ap()
    vin = nc.dram_tensor("vin", [A], f32, kind="ExternalInput").ap()
    maskin = nc.dram_tensor("maskin", [BC, S], i32, kind="ExternalInput").ap()
    ctx_out = nc.dram_tensor("ctx", [BC, E], f32, kind="ExternalOutput").ap()
    w_out = nc.dram_tensor("wout", [BC, S], f32, kind="ExternalOutput").ap()

    with tile.TileContext(nc) as tc:
        with ExitStack() as ctx:
            consts = ctx.enter_context(tc.tile_pool(name="consts", bufs=1))
            enc_pool = ctx.enter_context(tc.tile_pool(name="enc", bufs=3))
            encT_pool = ctx.enter_context(tc.tile_pool(name="encT", bufs=2))
            tanh_pool = ctx.enter_context(tc.tile_pool(name="tanh", bufs=4))
            scratch_pool = ctx.enter_context(tc.tile_pool(name="scr", bufs=3))
            small_pool = ctx.enter_context(tc.tile_pool(name="small", bufs=4))
            psumT_pool = ctx.enter_context(
                tc.tile_pool(name="psumT", bufs=3, space=PSUM)
            )
            psumS_pool = ctx.enter_context(
                tc.tile_pool(name="psumS", bufs=2, space=PSUM)
            )
            psumM_pool = ctx.enter_context(
                tc.tile_pool(name="psumM", bufs=1, space=PSUM)
            )

            # ---------------- setup (once) ----------------
            ident = consts.tile([P, P], f32)
            make_identity(nc, ident)
            ident_r = consts.tile([P, P], f32r)
            nc.vector.tensor_copy(ident_r, ident)

            ones_mat = consts.tile([P, P], f32)
            nc.vector.memset(ones_mat, 1.0)

            if stage >= 1:
                # v replicated across partitions (and twice along free): [128, 2, 256]
                v_rep_f = consts.tile([P, 4, A], f32)
                nc.gpsimd.dma_start(
                    out=v_rep_f,
                    in_=bass.AP(
                        tensor=vin.tensor, offset=0, ap=[[0, P], [0, 4], [1, A]]
                    ),
                )
                v_rep = consts.tile([P, 4, A], f16)
                nc.vector.tensor_copy(v_rep, v_rep_f)

                # W_enc^T in SBUF: [128(e_lo), 4(e_chunk), 256(a)]
                wenc_nat = consts.tile([P, 2, E], f32)
                nc.sync.dma_start(
                    out=wenc_nat, in_=wenc.rearrange("(ah p) e -> p ah e", p=P)
                )
                wencT = consts.tile([P, ET, A], f16)
                for ah in range(2):
                    for t in range(ET):
                        ps = psumM_pool.tile([P, P], f32, tag="psmisc")
                        nc.tensor.transpose(
                            ps, wenc_nat[:, ah, t * P : (t + 1) * P], ident
                        )
                        nc.vector.tensor_copy(
                            wencT[:, t, ah * P : (ah + 1) * P], ps
                        )

                # x correction, transposed to [128(e_lo), 4(e_chunk), 8(b)]
                x_nat = consts.tile([BC, E], f32)
                nc.sync.dma_start(out=x_nat, in_=xcor)
                xT = consts.tile([P, ET, BC], f32)
                for t in range(ET):
                    ps = psumM_pool.tile([P, BC], f32, tag="psmisc")
                    nc.tensor.transpose(
                        ps, x_nat[:, t * P : (t + 1) * P], ident[:BC, :BC]
                    )
                    nc.vector.tensor_copy(xT[:, t, :], ps)

                # mask -> additive -BIG mask, transposed to [128(s_lo), 16(j), 8(b)]
                mask_nat = consts.tile([BC, S], i32)
                nc.sync.dma_start(out=mask_nat, in_=maskin)
                mask_f = consts.tile([BC, S], f32)
                nc.vector.tensor_copy(mask_f, mask_nat)
                mask_m1 = consts.tile([BC, S], f32)
                nc.vector.tensor_scalar(
                    out=mask_m1,
                    in0=mask_f,
                    scalar1=1.0,
                    scalar2=1.0e30,
                    op0=mybir.AluOpType.subtract,
                    op1=mybir.AluOpType.mult,
                )
                maskT = consts.tile([P, SJ, BC], f32)
                for j in range(SJ):
                    ps = psumM_pool.tile([P, BC], f32, tag="psmisc")
                    nc.tensor.transpose(
                        ps, mask_m1[:, j * P : (j + 1) * P], ident[:BC, :BC]
                    )
                    nc.vector.tensor_copy(maskT[:, j, :], ps)

            # ---------------- per-batch pipeline ----------------
            # Front half (loads, transposes, scores matmuls, tanh, v-dot) of
            # batch b is emitted before the back half (softmax, context,
            # outputs) of batch b-1, so the PE's in-order stream never stalls
            # waiting on the exp->Z->context chain: the back-half matmuls of
            # b-1 sit behind the dense transpose/scores work of b.
            JQ = 4  # j-quad granularity for tanh / v-dot

            def emit_front(b):
                enc_sb = enc_pool.tile([P, SJ, E], f32r, tag="enc")
                csz = 2 if b == 0 else JG  # finer first-batch chunks: earlier start
                for c0 in range(0, SJ, csz):
                    nc.sync.dma_start(
                        out=enc_sb[:, c0 : c0 + csz, :],
                        in_=enc[b].rearrange("(j p) e -> p j e", p=P)[
                            :, c0 : c0 + csz, :
                        ],
                    )
                scores_sb = small_pool.tile([P, SJ], f32, tag="scores")

                # Phase 1: all 64 transposes of the batch, then Phase 2: all
                # 64 scores matmuls (this ordering benches faster than
                # interleaving per j-group)
                encT_b = encT_pool.tile([P, ET, SJ, P], f16, tag="encT")
                for jg in range(SJ // JG):
                    for t in range(ET):
                        psT = psumT_pool.tile([P, JG, P], f32r, tag="psT")
                        for jj in range(JG):
                            nc.tensor.transpose(
                                psT[:, jj, :],
                                enc_sb[:, jg * JG + jj, t * P : (t + 1) * P],
                                ident_r,
                            )
                        # evict + x-correction, rounding to fp16 so the scores
                        # matmuls get a 2-byte stationary (fast weight load)
                        if t >= 3:
                            nc.vector.tensor_scalar_add(
                                encT_b[:, t, jg * JG : (jg + 1) * JG, :],
                                psT,
                                xT[:, t, b : b + 1],
                            )
                        else:
                            nc.scalar.add(
                                encT_b[:, t, jg * JG : (jg + 1) * JG, :],
                                psT,
                                xT[:, t, b : b + 1],
                            )

                for jg in range(SJ // JG):
                    psS = psumS_pool.tile([P, JQ, A], f32, tag="psS")
                    for jl in range(JQ):
                        jglob = jg * JG + jl
                        for t in range(ET):
                            nc.tensor.matmul(
                                psS[:, jl, :],
                                lhsT=encT_b[:, t, jglob, :],
                                rhs=wencT[:, t, :],
                                start=(jl % 2 == 0 and t == 0),
                                stop=(jl % 2 == 1 and t == ET - 1),
                            )
                    tanh_sb = tanh_pool.tile([P, JQ, A], f16, tag="tanh")
                    nc.scalar.activation(tanh_sb, psS, Tanh)
                    jglob = jg * JG
                    scr = scratch_pool.tile([P, JQ, A], f16, tag="scr")
                    nc.vector.tensor_mul(scr, tanh_sb, v_rep)
                    nc.vector.tensor_reduce(
                        out=scores_sb[:, jglob : jglob + JQ],
                        in_=scr,
                        axis=mybir.AxisListType.X,
                        op=mybir.AluOpType.add,
                    )
                return enc_sb, scores_sb

            def emit_back(b, enc_sb, scores_sb):
                # mask + exp (+ per-partition partial sums)
                scores_m = small_pool.tile([P, SJ], f32, tag="scoresm")
                nc.vector.tensor_add(scores_m, scores_sb, maskT[:, :, b])
                exp_sb = small_pool.tile([P, SJ], f32r, tag="exp")
                zpart = small_pool.tile([P, 1], f32, tag="zpart")
                nc.scalar.activation(exp_sb, scores_m, Exp, accum_out=zpart)

                # Z = sum over partitions via all-ones matmul -> [128, 1]
                psZ = psumM_pool.tile([P, 1], f32, tag="psmisc")
                nc.tensor.matmul(psZ, lhsT=ones_mat, rhs=zpart, start=True, stop=True)
                recip_rep = small_pool.tile([P, 1], f32, tag="reciprep")
                nc.vector.reciprocal(recip_rep, psZ)

                # context = (sum_s w_un[s] * enc[s, :]) * recip
                psC = psumM_pool.tile([1, E], f32, tag="psmisc")
                for j in range(SJ):
                    nc.tensor.matmul(
                        psC,
                        lhsT=exp_sb[:, j : j + 1],
                        rhs=enc_sb[:, j, :],
                        start=(j == 0),
                        stop=(j == SJ - 1),
                    )
                ctx_sb = small_pool.tile([1, E], f32, tag="ctxsb")
                nc.vector.tensor_scalar_mul(ctx_sb, psC, recip_rep[0:1, :])
                nc.sync.dma_start(out=ctx_out[b : b + 1, :], in_=ctx_sb)

                # normalized weights, transposed back to s-major for output
                wn = small_pool.tile([P, SJ], f32, tag="wn")
                nc.vector.tensor_scalar_mul(wn, exp_sb, recip_rep)
                psW = psumM_pool.tile([SJ, P], f32, tag="psmisc")
                nc.tensor.transpose(psW, wn, ident)
                wt_sb = small_pool.tile([SJ, P], f32, tag="wt")
                nc.vector.tensor_copy(wt_sb, psW)
                nc.sync.dma_start(
                    out=w_out[b].rearrange("(j x) -> j x", x=P), in_=wt_sb
                )

            pend = None
            for b in range(BC):
                front = emit_front(b)
                if pend is not None:
                    emit_back(pend[0], *pend[1])
                pend = (b, front)
            emit_back(pend[0], *pend[1])

    nc.compile()
    return nc


def _get_program():
    stage = int(os.environ.get("KERNEL_STAGE", "99"))
    key = ("nc", stage)
    if key not in _cached:
        _cached[key] = _build_program(stage)
    return _cached[key]


def kernel(encoder_outputs, decoder_hidden, encoder_mask, W_enc, W_dec, v):
    # Recover wedged NeuronCores from any earlier crashed run.
    os.environ.setdefault("NEURON_RT_RESET_CORES", "1")
    from concourse.bass_utils import run_bass_kernel_spmd

    enc = np.ascontiguousarray(encoder_outputs, dtype=np.float32)
    dec = np.asarray(decoder_hidden, dtype=np.float32)
    mask = np.ascontiguousarray(encoder_mask, dtype=np.int32)
    we = np.ascontiguousarray(W_enc, dtype=np.float32)
    wd = np.asarray(W_dec, dtype=np.float32)
    vv = np.ascontiguousarray(v, dtype=np.float32)

    # host-side: dec_feat, then x = W_enc^+ dec_feat (exact: full row rank)
    dec_feat = dec.astype(np.float64) @ wd.astype(np.float64).T  # [B, A]
    G = we.astype(np.float64)
    try:
        x_all = (G.T @ np.linalg.solve(G @ G.T, dec_feat.T)).T  # [B, E]
    except np.linalg.LinAlgError:
        x_all = np.linalg.lstsq(G, dec_feat.T, rcond=None)[0].T
    x_all = np.ascontiguousarray(x_all, dtype=np.float32)

    nc = _get_program()
    in_maps = []
    for c in range(NCORES):
        sl = slice(c * BC, (c + 1) * BC)
        in_maps.append(
            {
                "enc": enc[sl],
                "wenc": we,
                "xcor": x_all[sl],
                "vin": vv,
                "maskin": mask[sl],
            }
        )

    trace = bool(int(os.environ.get("KERNEL_TRACE", "0")))
    res = run_bass_kernel_spmd(
        nc, in_maps, core_ids=list(range(NCORES)), trace=trace
    )
    _cached["last_results"] = res

    context = np.concatenate([r["ctx"] for r in res.results], axis=0)
    weights = np.concatenate([r["wout"] for r in res.results], axis=0)
    return context, weights
